# revision 1
# baseline (speedup 1.0000x reference)
"""DebertaV2 disentangled attention block on 8 TRN2 NeuronCores (Bass/Tile).

Head-sharded tensor parallel: 2 heads per core. Host does layout-only prep
(transpose / bucket-reversal / dtype cast); all FLOPs run on device.
ReduceScatter after the output dense; per-core LayerNorm on its 128 rows.
"""

import math

import numpy as np

H = 16
D = 64
HID = 1024
N = 1024
K = 1024
EPS = 1e-7
NCORES = 8
HPC = H // NCORES  # heads per core = 2
DPC = HPC * D      # head dims per core = 128
SCALE = 1.0 / math.sqrt(3.0 * D)  # applied inside exp()

W_WIN = 1151       # skew window width (127 + 1024)
P = 128

_CACHE = {}


def _build():
    import concourse.bass as bass
    import concourse.mybir as mybir
    import concourse.tile as tile
    from concourse import bacc
    from concourse.masks import make_identity
    from contextlib import ExitStack

    f32 = mybir.dt.float32
    bf16 = mybir.dt.bfloat16

    nc = bacc.Bacc(None, target_bir_lowering=False, debug=False)
    names = {}

    with tile.TileContext(nc) as tc, ExitStack() as es:
        dio = es.enter_context(tc.tile_pool(name="dram_io", bufs=1, space="DRAM"))
        dwork = es.enter_context(tc.tile_pool(name="dram_work", bufs=1, space="DRAM"))

        def din(nm, shape, dt=bf16):
            t = dio.tile(shape, dt, kind="ExternalInput", name=nm, tag=nm)
            names[nm] = t.name
            return t

        hsT = din("hsT", (HID, N))            # hs[0].T, bf16
        relTr = din("relTr", (HID, 2 * K))    # rel[::-1].T, bf16 (for pos_k)
        relTn = din("relTn", (HID, 2 * K))    # rel.T, bf16 (for pos_q)
        wqT = din("wqT", (HID, DPC))
        wkT = din("wkT", (HID, DPC))
        wvT = din("wvT", (HID, DPC))
        wpkT = din("wpkT", (HID, DPC))
        wpqT = din("wpqT", (HID, DPC))
        woT = din("woT", (DPC, HID))
        hs_rows = din("hs_rows", (P, HID), f32)
        bq_s = din("bq_s", (DPC,), f32)
        bk_s = din("bk_s", (DPC,), f32)
        bv_s = din("bv_s", (DPC,), f32)
        bpk_s = din("bpk_s", (DPC,), f32)
        bpq_s = din("bpq_s", (DPC,), f32)
        bo_t = din("bo", (HID,), f32)
        lng_t = din("ln_g", (HID,), f32)
        lnb_t = din("ln_b", (HID,), f32)

        out_t = dio.tile((P, HID), f32, kind="ExternalOutput", name="out", tag="out")
        names["out"] = out_t.name

        opart = dwork.tile((N, HID), bf16, name="opart", tag="opart")
        rs_out = dwork.tile((P, HID), bf16, name="rs_out", tag="rs_out")

        # ---- SBUF pools -------------------------------------------------
        wt = es.enter_context(tc.tile_pool(name="wt", bufs=1))
        work = es.enter_context(tc.tile_pool(name="work", bufs=1))
        psA = es.enter_context(tc.tile_pool(name="psA", bufs=6, space="PSUM"))
        psB = es.enter_context(tc.tile_pool(name="psB", bufs=1, space="PSUM"))

        Iden = mybir.ActivationFunctionType.Identity
        Exp = mybir.ActivationFunctionType.Exp
        Sqrt = mybir.ActivationFunctionType.Sqrt
        ADD = mybir.AluOpType.add
        MUL = mybir.AluOpType.mult
        SUB = mybir.AluOpType.subtract

        # ---- persistent small inputs ------------------------------------
        w_sb = {k: [] for k in ("q", "k", "v", "pk", "pq")}
        wmap = {"q": wqT, "k": wkT, "v": wvT, "pk": wpkT, "pq": wpqT}
        for t in range(8):
            for kk in w_sb:
                c = wt.tile([P, DPC], bf16, name=f"w{kk}{t}", tag=f"w{kk}{t}")
                nc.sync.dma_start(c[:], wmap[kk][128 * t:128 * (t + 1), :])
                w_sb[kk].append(c)
        woT_sb = wt.tile([P, HID], bf16, name="woT_sb", tag="woT_sb")
        nc.sync.dma_start(woT_sb[:], woT[:])

        def bias_tile(nm, src, n=DPC):
            t = wt.tile([n, 1], f32, name=nm, tag=nm)
            nc.sync.dma_start(t[:], bass.AP(src[:].tensor, src[:].offset, [[1, n]]))
            return t

        bq_sb = bias_tile("bq_sb", bq_s)
        bk_sb = bias_tile("bk_sb", bk_s)
        bv_sb = bias_tile("bv_sb", bv_s)
        bpk_sb = bias_tile("bpk_sb", bpk_s)
        bpq_sb = bias_tile("bpq_sb", bpq_s)

        def bcast_tile(nm, src, dt):
            t = wt.tile([P, HID], dt, name=nm, tag=nm)
            if dt == f32:
                nc.sync.dma_start(t[:], bass.AP(src[:].tensor, src[:].offset,
                                                [[0, P], [1, HID]]))
            else:
                nc.gpsimd.dma_start(t[:], bass.AP(src[:].tensor, src[:].offset,
                                                  [[0, P], [1, HID]]))
            return t

        bo_bc = bcast_tile("bo_bc", bo_t, f32)
        g_bc = bcast_tile("g_bc", lng_t, f32)
        b_bc = bcast_tile("b_bc", lnb_t, f32)

        hsr_sb = wt.tile([P, HID], f32, name="hsr_sb", tag="hsr_sb")
        nc.sync.dma_start(hsr_sb[:], hs_rows[:])

        ident = wt.tile([P, P], bf16, name="ident", tag="ident")
        make_identity(nc, ident[:])

        # ---- projections -------------------------------------------------
        qT = wt.tile([P, N], bf16, name="qT", tag="qT")
        kT = wt.tile([P, N], bf16, name="kT", tag="kT")
        pkT = wt.tile([P, 2 * K], bf16, name="pkT", tag="pkT")
        pqT = wt.tile([P, 2 * K], bf16, name="pqT", tag="pqT")

        def load_tiles(src, nt, width, nm):
            tiles, frees = [], []
            for t in range(nt):
                a, fa = tc.tile([P, width], bf16, name=f"{nm}{t}")
                nc.sync.dma_start(a[:], src[128 * t:128 * (t + 1), :])
                tiles.append(a)
                frees.append(fa)
            return tiles, frees

        def project(dst, wlist, rhs_list, width, bias):
            for c0 in range(0, width, 512):
                ps = psA.tile([P, 512], f32, name="pp", tag="pp")
                for t in range(8):
                    nc.tensor.matmul(ps[:], wlist[t][:],
                                     rhs_list[t][:, c0:c0 + 512],
                                     start=(t == 0), stop=(t == 7))
                nc.scalar.activation(dst[:, c0:c0 + 512], ps[:], Iden,
                                     bias=bias[:])

        hsT_sb, hsT_free = load_tiles(hsT, 8, N, "hsT")
        project(qT, w_sb["q"], hsT_sb, N, bq_sb)
        project(kT, w_sb["k"], hsT_sb, N, bk_sb)

        # v in [j, d] layout + ones column per head: va[jt] is [128, 132]
        va = []
        for jt in range(8):
            t = wt.tile([P, 132], bf16, name=f"va{jt}", tag=f"va{jt}")
            ps = psA.tile([P, DPC], f32, name="pv", tag="pp")
            for kt in range(8):
                nc.tensor.matmul(ps[:], hsT_sb[kt][:, 128 * jt:128 * (jt + 1)],
                                 w_sb["v"][kt][:], start=(kt == 0), stop=(kt == 7))
            nc.scalar.copy(t[:, 0:64], ps[:, 0:64])
            nc.scalar.copy(t[:, 66:130], ps[:, 64:128])
            nc.vector.memset(t[:, 64:65], 1.0)
            nc.vector.memset(t[:, 130:131], 1.0)
            va.append(t)
        for f in reversed(hsT_free):
            f()

        relTr_sb, relTr_free = load_tiles(relTr, 8, 2 * K, "relTr")
        project(pkT, w_sb["pk"], relTr_sb, 2 * K, bpk_sb)
        for f in reversed(relTr_free):
            f()
        relTn_sb, relTn_free = load_tiles(relTn, 8, 2 * K, "relTn")
        project(pqT, w_sb["pq"], relTn_sb, 2 * K, bpq_sb)
        for f in reversed(relTn_free):
            f()

        # ---- attention per head -----------------------------------------
        ctxT = wt.tile([P, N], bf16, name="ctxT", tag="ctxT")

        def skew_block(lhs, src_T, hd, idx, tagp, tag, bufs):
            """blk[p, c] = lhs[hd][:, 128*idx+p] . src_T[hd][:, w0+c]
            -> dst[p, x] = blk[p, 127 - p + x]   (shape [128, 1024])"""
            w0 = (896 if tagp == "c" else 897) - 128 * idx
            blk = work.tile([P, W_WIN], bf16, name=f"blk_{tagp}{idx}",
                            tag="blk", bufs=3)
            for (c0, w) in ((0, 512), (512, 512), (1024, 127)):
                ps = psA.tile([P, 512], f32, name="pblk", tag="pp")
                nc.tensor.matmul(
                    ps[:, 0:w],
                    lhs[hd, 128 * idx:128 * (idx + 1)],
                    src_T[hd, w0 + c0:w0 + c0 + w],
                    start=True, stop=True)
                if tagp == "c":
                    nc.vector.tensor_copy(blk[:, c0:c0 + w], ps[:, 0:w])
                else:
                    nc.scalar.copy(blk[:, c0:c0 + w], ps[:, 0:w])
            scr = dwork.tile((P * W_WIN,), bf16, name=f"scr_{tagp}{idx}",
                             tag="scr", bufs=4)
            h = scr[:].tensor
            nc.sync.dma_start(
                bass.AP(h, scr[:].offset, [[W_WIN, P], [1, W_WIN]]), blk[:])
            dst = work.tile([P, N], bf16, name=f"g_{tagp}{idx}", tag=tag,
                            bufs=bufs)
            nc.sync.dma_start(
                dst[:], bass.AP(h, scr[:].offset + 127, [[W_WIN - 1, P], [1, N]]))
            return dst

        for h in range(HPC):
            hd = slice(64 * h, 64 * h + 64)
            # c2p gathered tiles, one per i-tile r: [128 i, 1024 j]
            c2p = [skew_block(qT, pkT, hd, r, "c", f"g_c{r}", 1)
                   for r in range(8)]

            pb = psB.tile([65, N], f32, name="pb", tag="pb")
            for jt in range(8):
                # p2cT tile for this j-tile: [128 j, 1024 i]
                p2cT = skew_block(kT, pqT, hd, jt, "p", "g_p", 2)
                e = work.tile([P, N], bf16, name=f"expST{jt}", tag="expST",
                              bufs=2)
                for c in range(2):
                    st = psA.tile([P, 512], f32, name="st", tag="pp")
                    nc.tensor.matmul(st[:], kT[hd, 128 * jt:128 * (jt + 1)],
                                     qT[hd, 512 * c:512 * (c + 1)],
                                     start=True, stop=False)
                    for rr in range(4):
                        r = 4 * c + rr
                        nc.tensor.matmul(st[:, 128 * rr:128 * (rr + 1)],
                                         c2p[r][:, 128 * jt:128 * (jt + 1)],
                                         ident[:], start=False, stop=(rr == 3))
                    s_sb = work.tile([P, 512], f32, name="s_sb", tag="s_sb",
                                     bufs=3)
                    nc.vector.tensor_add(s_sb[:], st[:],
                                         p2cT[:, 512 * c:512 * (c + 1)])
                    nc.scalar.activation(e[:, 512 * c:512 * (c + 1)], s_sb[:],
                                         Exp, scale=SCALE)
                for c in range(2):
                    nc.tensor.matmul(pb[:, 512 * c:512 * (c + 1)],
                                     va[jt][:, 66 * h:66 * h + 65],
                                     e[:, 512 * c:512 * (c + 1)],
                                     start=(jt == 0), stop=(jt == 7))

            recip = work.tile([1, N], f32, name="recip", tag="recip", bufs=2)
            nc.vector.reciprocal(recip[:], pb[64:65, :])
            rscr = dwork.tile((N,), f32, name=f"rscr{h}", tag="rscr", bufs=2)
            rh = rscr[:].tensor
            nc.sync.dma_start(bass.AP(rh, rscr[:].offset, [[1, N]]), recip[:])
            rbc = work.tile([64, N], f32, name="rbc", tag="rbc", bufs=2)
            nc.sync.dma_start(rbc[:], bass.AP(rh, rscr[:].offset, [[0, 64], [1, N]]))
            ctmp = work.tile([64, N], bf16, name="ctmp", tag="ctmp", bufs=2)
            nc.vector.tensor_mul(ctmp[:], pb[0:64, :], rbc[:])
            nc.scalar.activation(ctxT[hd, :], ctmp[:], Iden, bias=bv_sb[hd, :])

        # ---- output dense (partial) -> DRAM ------------------------------
        for it in range(8):
            osb = work.tile([P, HID], bf16, name="osb", tag="osb", bufs=2)
            for c in range(2):
                po = psA.tile([P, 512], f32, name="po", tag="pp")
                nc.tensor.matmul(po[:], ctxT[:, 128 * it:128 * (it + 1)],
                                 woT_sb[:, 512 * c:512 * (c + 1)],
                                 start=True, stop=True)
                nc.scalar.copy(osb[:, 512 * c:512 * (c + 1)], po[:])
            nc.sync.dma_start(opart[128 * it:128 * (it + 1), :], osb[:])

        # ---- ReduceScatter ------------------------------------------------
        nc.gpsimd.collective_compute(
            "ReduceScatter", ADD, replica_groups=[list(range(NCORES))],
            ins=[opart[:]], outs=[rs_out[:]])

        # ---- residual + LayerNorm on this core's 128 rows ----------------
        xr = wt.tile([P, HID], f32, name="xr", tag="xr")
        nc.gpsimd.dma_start(xr[:], rs_out[:])  # bf16 -> f32 cast dma
        x = wt.tile([P, HID], f32, name="x", tag="x")
        nc.vector.tensor_add(x[:], xr[:], hsr_sb[:])
        nc.vector.tensor_add(x[:], x[:], bo_bc[:])

        stats = wt.tile([P, 2, 6], f32, name="stats", tag="stats")
        mv = wt.tile([P, 2], f32, name="mv", tag="mv")
        for s in range(2):
            nc.vector.bn_stats(stats[:, s, :], x[:, 512 * s:512 * (s + 1)])
        nc.vector.bn_aggr(mv[:], stats[:])
        epsb = wt.tile([P, 1], f32, name="epsb", tag="epsb")
        nc.vector.memset(epsb[:], EPS)
        std = wt.tile([P, 1], f32, name="std", tag="std")
        nc.scalar.activation(std[:], mv[:, 1:2], Sqrt, bias=epsb[:])
        rstd = wt.tile([P, 1], f32, name="rstd", tag="rstd")
        nc.vector.reciprocal(rstd[:], std[:])

        t1 = wt.tile([P, HID], f32, name="t1", tag="t1")
        nc.vector.scalar_tensor_tensor(t1[:], x[:], mv[:, 0:1], g_bc[:],
                                       op0=SUB, op1=MUL)
        yout = wt.tile([P, HID], f32, name="yout", tag="yout")
        nc.vector.scalar_tensor_tensor(yout[:], t1[:], rstd[:], b_bc[:],
                                       op0=MUL, op1=ADD)
        nc.sync.dma_start(out_t[:], yout[:])

    nc.compile()
    return nc, names


def _get_compiled():
    if "nc" not in _CACHE:
        nc, names = _build()
        _CACHE["nc"] = nc
        _CACHE["names"] = names
    return _CACHE["nc"], _CACHE["names"]


def _prep_in_maps(inputs):
    import ml_dtypes

    bf = ml_dtypes.bfloat16
    hs = np.asarray(inputs["hidden_states"], np.float32)[0]      # (N, HID)
    rel = np.asarray(inputs["rel_embeddings"], np.float32)       # (2K, HID)
    hsT = np.ascontiguousarray(hs.T).astype(bf)
    relTr = np.ascontiguousarray(rel[::-1].T).astype(bf)
    relTn = np.ascontiguousarray(rel.T).astype(bf)

    def wT(w, r):
        w = np.asarray(w, np.float32)
        return np.ascontiguousarray(w[DPC * r:DPC * (r + 1), :].T).astype(bf)

    in_maps = []
    for r in range(NCORES):
        m = {
            "hsT": hsT,
            "relTr": relTr,
            "relTn": relTn,
            "wqT": wT(inputs["Wq"], r),
            "wkT": wT(inputs["Wk"], r),
            "wvT": wT(inputs["Wv"], r),
            "wpkT": wT(inputs["Wpk"], r),
            "wpqT": wT(inputs["Wpq"], r),
            "woT": np.ascontiguousarray(
                np.asarray(inputs["Wo"], np.float32)[:, DPC * r:DPC * (r + 1)].T
            ).astype(bf),
            "hs_rows": np.ascontiguousarray(hs[P * r:P * (r + 1), :]),
            "bq_s": np.asarray(inputs["bq"], np.float32)[DPC * r:DPC * (r + 1)],
            "bk_s": np.asarray(inputs["bk"], np.float32)[DPC * r:DPC * (r + 1)],
            "bv_s": np.asarray(inputs["bv"], np.float32)[DPC * r:DPC * (r + 1)],
            "bpk_s": np.asarray(inputs["bpk"], np.float32)[DPC * r:DPC * (r + 1)],
            "bpq_s": np.asarray(inputs["bpq"], np.float32)[DPC * r:DPC * (r + 1)],
            "bo": np.asarray(inputs["bo"], np.float32),
            "ln_g": np.asarray(inputs["ln_g"], np.float32),
            "ln_b": np.asarray(inputs["ln_b"], np.float32),
        }
        in_maps.append(m)
    return in_maps


def run(inputs, trace=False):
    from concourse.bass_utils import run_bass_kernel_spmd

    nc, names = _get_compiled()
    logical = _prep_in_maps(inputs)
    in_maps = [{names[k]: v for k, v in m.items()} for m in logical]
    res = run_bass_kernel_spmd(nc, in_maps, list(range(NCORES)), trace=trace)
    outs = [res.results[r][names["out"]].astype(np.float32) for r in range(NCORES)]
    full = np.concatenate(outs, axis=0).reshape(1, N, HID)
    return full, res


def kernel(**inputs) -> np.ndarray:
    full, _ = run(inputs, trace=False)
    return full



# revision 41
# speedup vs baseline: 2.5869x; 2.5869x over previous
"""DebertaV2 disentangled attention block on 8 TRN2 NeuronCores (Bass/Tile).

Head-sharded tensor parallel: 2 heads per core. Host does layout-only prep
(transpose / bucket-reversal / dtype cast); all FLOPs run on device.
ReduceScatter after the output dense; per-core LayerNorm on its 128 rows.

Perf notes (cost-model driven):
- All HBM loads are single batched DMAs with multi-dim access patterns
  (per-dma fixed cost ~1.2us serialized on the issuing queue + HWDGE).
- Skew gather (c2p/p2c band -> per-row shifted read) is batched 4 tiles
  per DMA through a DRAM scratch with row stride W_WIN-1.
- Skew block matmuls write one 3-bank PSUM tile -> single PSUM->SBUF copy.
- Softmax reciprocal is broadcast across partitions with a K=1 matmul
  instead of a DRAM roundtrip.
- DMA issue spread across SP (sync), ACT (scalar) and Pool (gpsimd).
"""

import math

import numpy as np

H = 16
D = 64
HID = 1024
N = 1024
K = 1024
EPS = 1e-7
NCORES = 8
HPC = H // NCORES  # heads per core = 2
DPC = HPC * D      # head dims per core = 128
SCALE = 1.0 / math.sqrt(3.0 * D)  # applied inside exp()

W_WIN = 1151       # skew window width (127 + 1024)
SCR_STRIDE = 128 * W_WIN
P = 128

_CACHE = {}


def _build():
    import concourse.bass as bass
    import concourse.mybir as mybir
    import concourse.tile as tile
    from concourse import bacc
    from concourse.masks import make_identity
    from contextlib import ExitStack

    f32 = mybir.dt.float32
    bf16 = mybir.dt.bfloat16
    fp8 = mybir.dt.float8e4

    nc = bacc.Bacc(None, target_bir_lowering=False, debug=False)
    names = {}

    with tile.TileContext(nc) as tc, ExitStack() as es:
        dio = es.enter_context(tc.tile_pool(name="dram_io", bufs=1, space="DRAM"))
        dwork = es.enter_context(tc.tile_pool(name="dram_work", bufs=1, space="DRAM"))

        def din(nm, shape, dt=bf16):
            t = dio.tile(shape, dt, kind="ExternalInput", name=nm, tag=nm)
            names[nm] = t.name
            return t

        hsT = din("hsT", (HID, N))            # hs[0].T, bf16
        relTn = din("relTn", (HID, 2 * K), fp8)  # rel.T, fp8 (pos_q; pos_k
        #                                       streams it column-reversed)
        wqT = din("wqT", (HID, DPC))
        wkT = din("wkT", (HID, DPC))
        wvT = din("wvT", (HID, DPC))
        wpkT = din("wpkT", (HID, DPC), fp8)   # prescaled x16 on host
        wpqT = din("wpqT", (HID, DPC), fp8)   # prescaled x16 on host
        woT = din("woT", (HID, HID))          # full Wo^T (dense runs
        #                                       post-AllToAll on own rows)
        hs_rows = din("hs_rows", (P, HID), f32)
        b5 = din("b5", (5, DPC), f32)         # bq|bk|bv|bpk|bpq (per-core slice)
        b3 = din("b3", (3, HID), f32)         # bo|ln_g|ln_b

        out_t = dio.tile((P, HID), f32, kind="ExternalOutput", name="out", tag="out")
        names["out"] = out_t.name

        # AllToAll buffers: shard j = my ctx block [128 dpc, 128 i] for core j
        ctx_send = dwork.tile((NCORES * P * P,), bf16, name="ctx_send",
                              tag="ctx_send")
        ctx_rcv = dwork.tile((NCORES * P * P,), bf16, name="ctx_rcv",
                             tag="ctx_rcv")

        # ---- SBUF pools -------------------------------------------------
        wt = es.enter_context(tc.tile_pool(name="wt", bufs=1))
        work = es.enter_context(tc.tile_pool(name="work", bufs=1))
        psS = es.enter_context(tc.tile_pool(name="psS", bufs=2, space="PSUM"))
        psB = es.enter_context(tc.tile_pool(name="psB", bufs=1, space="PSUM"))
        psK = es.enter_context(tc.tile_pool(name="psK", bufs=4, space="PSUM"))

        Iden = mybir.ActivationFunctionType.Identity
        Exp = mybir.ActivationFunctionType.Exp
        Sqrt = mybir.ActivationFunctionType.Sqrt
        ADD = mybir.AluOpType.add
        MUL = mybir.AluOpType.mult
        SUB = mybir.AluOpType.subtract

        # ---- upfront batched loads --------------------------------------
        # Spread across SP/ACT/Pool queues so the streams run concurrently;
        # the pos-projection inputs (rel + wpk/wpq) gate the longest chain.
        w_all = {}

        def load_w(kk, src, eng, dt=bf16):
            t = wt.tile([P, 8 * DPC], dt, name=f"w{kk}", tag=f"w{kk}")
            eng.dma_start(
                t[:], bass.AP(src[:].tensor, src[:].offset,
                              [[DPC, P], [P * DPC, 8], [1, DPC]]))
            w_all[kk] = t

        load_w("pk", wpkT, nc.sync, fp8)
        load_w("pq", wpqT, nc.sync, fp8)

        # hidden states at the bottom of the stack (freed last)
        hsT_all, hsT_free = tc.tile([P, 8 * N], bf16, name="hsT_all")
        rn0, rn0_f = tc.tile([P, 4 * 2 * K], fp8, name="relTn0")
        rn1, rn1_f = tc.tile([P, 4 * 2 * K], fp8, name="relTn1")
        nc.gpsimd.dma_start(
            hsT_all[:], bass.AP(hsT[:].tensor, hsT[:].offset,
                                [[N, P], [P * N, 8], [1, N]]))

        # rel embedding halves as [128 hid-k, 4 tiles x 2048 pos]
        for half, t, eng in ((0, rn0, nc.sync), (1, rn1, nc.scalar)):
            base = relTn[:].offset + half * 4 * P * 2 * K
            eng.dma_start(
                t[:], bass.AP(relTn[:].tensor, base,
                              [[2 * K, P], [P * 2 * K, 4], [1, 2 * K]]))

        load_w("q", wqT, nc.sync)
        load_w("k", wkT, nc.scalar)
        load_w("v", wvT, nc.gpsimd)

        b5_sb = wt.tile([P, 5], f32, name="b5_sb", tag="b5_sb")
        nc.sync.dma_start(
            b5_sb[:], bass.AP(b5[:].tensor, b5[:].offset, [[1, P], [P, 5]]))

        ident = wt.tile([P, P], bf16, name="ident", tag="ident")
        make_identity(nc, ident[:])
        id8 = wt.tile([P, P], fp8, name="id8", tag="id8")
        nc.scalar.copy(id8[:], ident[:])
        ones1 = wt.tile([1, 64], f32, name="ones1", tag="ones1")
        nc.vector.memset(ones1[:], 1.0)

        # late loads (only needed at the output stage)
        woT_sb = wt.tile([P, 8 * HID], bf16, name="woT_sb", tag="woT_sb")
        nc.scalar.dma_start(
            woT_sb[:], bass.AP(woT[:].tensor, woT[:].offset,
                               [[HID, P], [P * HID, 8], [1, HID]]))
        hsr_sb = wt.tile([P, HID], f32, name="hsr_sb", tag="hsr_sb")
        nc.gpsimd.dma_start(hsr_sb[:], hs_rows[:])
        bc_all = wt.tile([P, 3 * HID], f32, name="bc_all", tag="bc_all")
        nc.gpsimd.dma_start(
            bc_all[:], bass.AP(b3[:].tensor, b3[:].offset, [[0, P], [1, 3 * HID]]))
        # hs residual + bo, precomputed off the post-collective tail
        hsb = wt.tile([P, HID], f32, name="hsb", tag="hsb")
        nc.vector.tensor_add(hsb[:], hsr_sb[:], bc_all[:, 0:HID])

        # ---- projections -------------------------------------------------
        qT = wt.tile([P, N], bf16, name="qT", tag="qT")
        kT = wt.tile([P, N], bf16, name="kT", tag="kT")
        pkT = wt.tile([P, 2 * K], bf16, name="pkT", tag="pkT")
        pqT = wt.tile([P, 2 * K], bf16, name="pqT", tag="pqT")

        DblRow = mybir.MatmulPerfMode.DoubleRow
        PDESC = 1.0 / 16.0  # undo the x16 host prescale of fp8 pos weights

        # pos_k projection first (it gates the skew chains): stream relTn
        # column-REVERSED so pkT comes out in the bucket-reversed layout the
        # c2p skew gather needs. fp8 + DoubleRow, 2x PE throughput.
        rel_w = 4 * 2 * K  # free width of a rel half tile

        def rel_rev_rhs(rn, t4, c0):
            full = rn[:]
            off = full.offset + 2 * K * t4 + (2 * K - 1) - c0
            return bass.AP(full.tensor, off, [[rel_w, P], [-1, 512]])

        def w_pair(w, j):
            # [K, 2 k-tiles, M] stationary pair for DoubleRow
            full = w[:]
            return bass.AP(full.tensor, full.offset + 2 * DPC * j,
                           [[8 * DPC, P], [DPC, 2], [1, DPC]])

        def rel_pair(j, c0, rev):
            # [K, 2 k-tiles, 512] moving pair; k-tile pairs never straddle
            # the two rel halves
            rn = rn0 if j < 2 else rn1
            full = rn[:]
            t4 = 2 * (j % 2)
            if rev:
                off = full.offset + 2 * K * t4 + (2 * K - 1) - c0
                return bass.AP(full.tensor, off,
                               [[rel_w, P], [2 * K, 2], [-1, 512]])
            off = full.offset + 2 * K * t4 + c0
            return bass.AP(full.tensor, off,
                           [[rel_w, P], [2 * K, 2], [1, 512]])

        for c0 in range(0, 2 * K, 512):
            ps = psS.tile([P, 512], f32, name="ppk", tag="ps512", bufs=2)
            for j in range(4):
                nc.tensor.matmul(ps[:], w_pair(w_all["pk"], j),
                                 rel_pair(j, c0, True),
                                 start=(j == 0), stop=(j == 3),
                                 perf_mode=DblRow)
            # reversed stream: output col j of this chunk is pos 2047-c0-j,
            # i.e. pkT[:, c] = pos_k[2047-c]
            nc.scalar.activation(pkT[:, c0:c0 + 512], ps[:], Iden,
                                 bias=b5_sb[:, 3:4], scale=PDESC)

        def project(dst, w, src_all, tilew, bcol):
            for c0 in range(0, tilew, 512):
                ps = psS.tile([P, 512], f32, name="pp", tag="ps512", bufs=2)
                for t in range(8):
                    nc.tensor.matmul(ps[:], w[:, DPC * t:DPC * (t + 1)],
                                     src_all[:, tilew * t + c0:tilew * t + c0 + 512],
                                     start=(t == 0), stop=(t == 7))
                nc.scalar.activation(dst[:, c0:c0 + 512], ps[:], Iden, bias=bcol)

        # qT right after pkT: together they unblock the first skew chain
        project(qT, w_all["q"], hsT_all, N, b5_sb[:, 0:1])

        for c0 in range(0, 2 * K, 512):
            ps = psS.tile([P, 512], f32, name="ppq", tag="ps512", bufs=2)
            for j in range(4):
                nc.tensor.matmul(ps[:], w_pair(w_all["pq"], j),
                                 rel_pair(j, c0, False),
                                 start=(j == 0), stop=(j == 3),
                                 perf_mode=DblRow)
            nc.scalar.activation(pqT[:, c0:c0 + 512], ps[:], Iden,
                                 bias=b5_sb[:, 4:5], scale=PDESC)
        rn1_f()
        rn0_f()

        project(kT, w_all["k"], hsT_all, N, b5_sb[:, 1:2])

        # v in [j, d] layout + ones column per head: va[jt] is [128, 132]
        va = []
        for jt in range(8):
            t = wt.tile([P, 132], bf16, name=f"va{jt}", tag=f"va{jt}")
            ps = psS.tile([P, DPC], f32, name="pv", tag="ps512", bufs=2)
            for kt in range(8):
                nc.tensor.matmul(ps[:], hsT_all[:, N * kt + P * jt:N * kt + P * (jt + 1)],
                                 w_all["v"][:, DPC * kt:DPC * (kt + 1)],
                                 start=(kt == 0), stop=(kt == 7))
            nc.vector.tensor_copy(t[:, 0:64], ps[:, 0:64])
            nc.vector.tensor_copy(t[:, 66:130], ps[:, 64:128])
            nc.vector.memset(t[:, 64:65], 1.0)
            nc.vector.memset(t[:, 130:131], 1.0)
            va.append(t)
        hsT_free()

        # ---- attention per head -----------------------------------------
        ctxT = wt.tile([P, N], bf16, name="ctxT", tag="ctxT")

        copy_flip = [0]

        def skew_batch(lhs, src_T, hd, tagp, g):
            """g[p, r*1024 + x] = lhs[hd][:, 128r+p] . src_T[hd][:, w0_r + 127-p+x]

            Band matmuls in 1-bank PSUM chunks (rotating 4-slot pool, fp8
            SBUF staging, copies split across DVE/ACT) -> 4-block batched
            DRAM roundtrip with row stride W_WIN-1 (the per-partition
            diagonal shift)."""
            for half in (0, 1):
                blk, blk_f = tc.tile([P, 4 * W_WIN], fp8, name=f"blk_{tagp}{half}")
                for ri in range(4):
                    r = half * 4 + ri
                    w0 = (896 if tagp == "c" else 897) - 128 * r
                    for (c0, w) in ((0, 512), (512, 512), (1024, 127)):
                        ps = psK.tile([P, 512], f32, name="bps", tag="bps",
                                      bufs=4)
                        nc.tensor.matmul(
                            ps[:, 0:w],
                            lhs[hd, 128 * r:128 * (r + 1)],
                            src_T[hd, w0 + c0:w0 + c0 + w],
                            start=True, stop=True)
                        dst = blk[:, W_WIN * ri + c0:W_WIN * ri + c0 + w]
                        # balance PSUM->SBUF copies: A->DVE, B->ACT, C splits
                        if c0 == 0:
                            use_dve = True
                        elif c0 == 512:
                            use_dve = False
                        else:
                            use_dve = copy_flip[0] % 2 == 0
                            copy_flip[0] += 1
                        if use_dve:
                            nc.vector.tensor_copy(dst, ps[:, 0:w])
                        else:
                            nc.scalar.copy(dst, ps[:, 0:w])
                scr = dwork.tile((4 * P * W_WIN,), fp8, name=f"scr_{tagp}{half}",
                                 tag="scr", bufs=4)
                hdl = scr[:].tensor
                base = scr[:].offset
                nc.sync.dma_start(
                    bass.AP(hdl, base, [[W_WIN, P], [SCR_STRIDE, 4], [1, W_WIN]]),
                    blk[:])
                blk_f()
                nc.sync.dma_start(
                    g[:, half * 4096:(half + 1) * 4096],
                    bass.AP(hdl, base + 127,
                            [[W_WIN - 1, P], [SCR_STRIDE, 4], [1, N]]))

        gs = []
        for h in range(HPC):
            hd = slice(64 * h, 64 * h + 64)
            # c2p gathered: i-tile r at cols [r*1024, (r+1)*1024), [i, j] layout
            gc, gc_f = tc.tile([P, 8 * N], fp8, name=f"g_c{h}")
            skew_batch(qT, pkT, hd, "c", gc)
            # p2cT gathered: j-tile jt at cols [jt*1024, ...), [j, i] layout
            gp, gp_f = tc.tile([P, 8 * N], fp8, name=f"g_p{h}")
            skew_batch(kT, pqT, hd, "p", gp)
            gs.append((gc, gc_f, gp, gp_f))

        for h in range(HPC):
            hd = slice(64 * h, 64 * h + 64)
            gc, _, gp, _ = gs[h]

            pb = psB.tile([65, N], f32, name="pb", tag="pb", bufs=1)
            for jt in range(8):
                e = work.tile([P, N], bf16, name=f"expST{jt}", tag="expST",
                              bufs=2)
                for c in range(2):
                    st = psS.tile([P, 512], f32, name="st", tag="ps512", bufs=2)
                    nc.tensor.matmul(st[:], kT[hd, 128 * jt:128 * (jt + 1)],
                                     qT[hd, 512 * c:512 * (c + 1)],
                                     start=True, stop=False)
                    for rr in range(4):
                        r = 4 * c + rr
                        nc.tensor.matmul(
                            st[:, 128 * rr:128 * (rr + 1)],
                            gc[:, N * r + 128 * jt:N * r + 128 * (jt + 1)],
                            id8[:], start=False, stop=False)
                    # p2cT folded into the PSUM accumulation (id8.T @ gp = gp)
                    nc.tensor.matmul(
                        st[:], id8[:],
                        gp[:, N * jt + 512 * c:N * jt + 512 * (c + 1)],
                        start=False, stop=True)
                    nc.scalar.activation(e[:, 512 * c:512 * (c + 1)], st[:],
                                         Exp, scale=SCALE)
                for c in range(2):
                    nc.tensor.matmul(pb[:, 512 * c:512 * (c + 1)],
                                     va[jt][:, 66 * h:66 * h + 65],
                                     e[:, 512 * c:512 * (c + 1)],
                                     start=(jt == 0), stop=(jt == 7))

            # softmax normalize; reciprocal broadcast across partitions via
            # a K=1 matmul (outer product with a ones column)
            recip = work.tile([1, N], f32, name="recip", tag="recip", bufs=2)
            nc.vector.reciprocal(recip[:], pb[64:65, :])
            ctmp = work.tile([64, N], bf16, name="ctmp", tag="ctmp", bufs=2)
            rbs = work.tile([64, N], f32, name="rbs", tag="rbs", bufs=2)
            for c in range(2):
                rb = psS.tile([64, 512], f32, name="rb", tag="ps512", bufs=2)
                nc.tensor.matmul(rb[:], ones1[:], recip[:, 512 * c:512 * (c + 1)],
                                 start=True, stop=True)
                # DVE can read only one PSUM operand; stage rb in SBUF
                nc.scalar.copy(rbs[:, 512 * c:512 * (c + 1)], rb[:])
                nc.vector.tensor_mul(ctmp[:, 512 * c:512 * (c + 1)],
                                     pb[0:64, 512 * c:512 * (c + 1)],
                                     rbs[:, 512 * c:512 * (c + 1)])
            nc.scalar.activation(ctxT[hd, :], ctmp[:], Iden, bias=b5_sb[hd, 2:3])

        for gc, gc_f, gp, gp_f in reversed(gs):
            gp_f()
            gc_f()

        # ---- AllToAll of per-head-normalized context ---------------------
        # shard j (contiguous 32KB) = my [128 dpc, 128 i] block for core j;
        # after A2A, block j' = core j's dpc dims for MY 128 rows.
        csh = ctx_send[:].tensor
        csb = ctx_send[:].offset
        nc.sync.dma_start(
            bass.AP(csh, csb, [[P, P], [P * P, NCORES], [1, P]]), ctxT[:])
        nc.gpsimd.collective_compute(
            "AllToAll", mybir.AluOpType.bypass,
            replica_groups=[list(range(NCORES))],
            ins=[ctx_send[:]], outs=[ctx_rcv[:]])
        ctx_sb = wt.tile([P, 8 * P], bf16, name="ctx_sb", tag="ctx_sb")
        crh = ctx_rcv[:].tensor
        crb = ctx_rcv[:].offset
        nc.sync.dma_start(
            ctx_sb[:], bass.AP(crh, crb, [[P, P], [P * P, NCORES], [1, P]]))

        # ---- output dense on own 128 rows + residual ---------------------
        x = wt.tile([P, HID], f32, name="x", tag="x")
        for c in range(2):
            po = psS.tile([P, 512], f32, name="po", tag="ps512", bufs=2)
            for j in range(8):
                nc.tensor.matmul(po[:], ctx_sb[:, P * j:P * (j + 1)],
                                 woT_sb[:, HID * j + 512 * c:
                                        HID * j + 512 * (c + 1)],
                                 start=(j == 0), stop=(j == 7))
            # residual + bo folded in from the precomputed hsb tile
            nc.vector.tensor_add(x[:, 512 * c:512 * (c + 1)], po[:],
                                 hsb[:, 512 * c:512 * (c + 1)])

        stats = wt.tile([P, 2, 6], f32, name="stats", tag="stats")
        mv = wt.tile([P, 2], f32, name="mv", tag="mv")
        for s in range(2):
            nc.vector.bn_stats(stats[:, s, :], x[:, 512 * s:512 * (s + 1)])
        nc.vector.bn_aggr(mv[:], stats[:])
        epsb = wt.tile([P, 1], f32, name="epsb", tag="epsb")
        nc.vector.memset(epsb[:], EPS)
        std = wt.tile([P, 1], f32, name="std", tag="std")
        nc.scalar.activation(std[:], mv[:, 1:2], Sqrt, bias=epsb[:])
        rstd = wt.tile([P, 1], f32, name="rstd", tag="rstd")
        nc.vector.reciprocal(rstd[:], std[:])

        t1 = wt.tile([P, HID], f32, name="t1", tag="t1")
        nc.vector.scalar_tensor_tensor(t1[:], x[:], mv[:, 0:1],
                                       bc_all[:, HID:2 * HID],
                                       op0=SUB, op1=MUL)
        yout = wt.tile([P, HID], f32, name="yout", tag="yout")
        nc.vector.scalar_tensor_tensor(yout[:], t1[:], rstd[:],
                                       bc_all[:, 2 * HID:3 * HID],
                                       op0=MUL, op1=ADD)
        nc.sync.dma_start(out_t[:], yout[:])

    nc.compile()
    return nc, names


def _get_compiled():
    if "nc" not in _CACHE:
        nc, names = _build()
        _CACHE["nc"] = nc
        _CACHE["names"] = names
    return _CACHE["nc"], _CACHE["names"]


def _prep_in_maps(inputs):
    import ml_dtypes

    bf = ml_dtypes.bfloat16
    f8 = ml_dtypes.float8_e4m3
    hs = np.asarray(inputs["hidden_states"], np.float32)[0]      # (N, HID)
    rel = np.asarray(inputs["rel_embeddings"], np.float32)       # (2K, HID)
    hsT = np.ascontiguousarray(hs.T).astype(bf)
    relTn = np.ascontiguousarray(rel.T).astype(f8)
    b3 = np.ascontiguousarray(np.stack([
        np.asarray(inputs["bo"], np.float32),
        np.asarray(inputs["ln_g"], np.float32),
        np.asarray(inputs["ln_b"], np.float32),
    ]))
    woTf = np.ascontiguousarray(np.asarray(inputs["Wo"], np.float32).T).astype(bf)

    def wT(w, r, dt=bf, scale=1.0):
        w = np.asarray(w, np.float32) * scale
        return np.ascontiguousarray(w[DPC * r:DPC * (r + 1), :].T).astype(dt)

    in_maps = []
    for r in range(NCORES):
        b5 = np.ascontiguousarray(np.stack([
            np.asarray(inputs[k], np.float32)[DPC * r:DPC * (r + 1)]
            for k in ("bq", "bk", "bv", "bpk", "bpq")
        ]))
        m = {
            "hsT": hsT,
            "relTn": relTn,
            "wqT": wT(inputs["Wq"], r),
            "wkT": wT(inputs["Wk"], r),
            "wvT": wT(inputs["Wv"], r),
            "wpkT": wT(inputs["Wpk"], r, f8, 16.0),
            "wpqT": wT(inputs["Wpq"], r, f8, 16.0),
            "woT": woTf,
            "hs_rows": np.ascontiguousarray(hs[P * r:P * (r + 1), :]),
            "b5": b5,
            "b3": b3,
        }
        in_maps.append(m)
    return in_maps


def run(inputs, trace=False):
    from concourse.bass_utils import run_bass_kernel_spmd

    nc, names = _get_compiled()
    logical = _prep_in_maps(inputs)
    in_maps = [{names[k]: v for k, v in m.items()} for m in logical]
    res = run_bass_kernel_spmd(nc, in_maps, list(range(NCORES)), trace=trace)
    outs = [res.results[r][names["out"]].astype(np.float32) for r in range(NCORES)]
    full = np.concatenate(outs, axis=0).reshape(1, N, HID)
    return full, res


def kernel(**inputs) -> np.ndarray:
    full, _ = run(inputs, trace=False)
    return full
